# revision 27
# baseline (speedup 1.0000x reference)
"""Trainium2 Bass kernel for nn_AiMAiPartiallyConnectedLayers.

26 independent MLPs (5 -> 64 -> 64 -> 1, tanh) applied per node type over a
batch of 65536 samples; output [B, 26] fp32.  Pure data parallel over 8
NeuronCores (8192 samples each); ~255 us HW time, rel err ~3.6e-3 (bf16
matmul precision).

Design (ScalarE/tanh is the bottleneck engine, ~87% busy):
  - Types in 13 pairs, packed block-diagonal [128, 128] bf16 per pair.
    Input pre-transposed host-side to xt[tile, q, 128, 512]; plane q holds
    pairs 4q+k on partition rows 32k:32k+11 (10 channel rows + a ones row
    folding b1).  Layer-1 matmuls: K=32 with row tile_position (32k, 0);
    the 4 matmuls of a plane co-issue on disjoint PE row-quadrants.
  - ACT1 is one op per PLANE over a 4-bank PSUM region [128, 4, 512]
    (2048 cols), amortizing the ~350-420ns/op ACT fixed cost (marginal is
    ~0.49ns/col).  ps1 is a single 4-bank buffer; refills hide under the
    ACT2 drains between ACT1 ops.
  - ACT2 per pair (bias b2 via the ACT per-partition bias port; a bias
    port column cannot span pairs, which pins ACT2 at 512 cols).
  - Layer-3 weights for pair p have nonzero columns 2p, 2p+1 inside the
    pair block, so ALL 13 pairs accumulate into rows 0:26 of ONE PSUM bank
    (start/stop flags).  One DVE copy moves [26, 512] to SBUF, one DMA
    writes it to a type-major DRAM output [26, bc]; the HOST transposes
    and adds b3 (kills the v1 gather-DMA + PE-transpose tail).
  - Slot pipeline over (tile, plane): ACT1(q) | L3s(lag-1) | 2 drains
    (L2+ACT2) | L1(next plane) | remaining drains.  A FIFO with per-plane
    ready keys drains pairs one plane late (no lag on the last tile), so
    ScalarE never waits for ps1 refills or L2 matmuls.
  - PSUM: ps1 4 banks (single) + ps2 2x1 + ps3 2x1 = 8 banks exactly.

Notes for future edits (hard-won):
  - Build with bacc.Bacc, not bass.Bass: Bacc.finalize() legalizes multi-
    semaphore waits (walrus allows ONE embedded wait per instruction).
  - tile_position column offsets fail walrus' ISA check; only row tiling.
    Matmul lhsT/rhs base partition must be 0/32/64/96; PSUM-dst matmuls
    must fit ONE 2KB bank (N <= 512 fp32).  With K < 32 the PE still
    streams the full 32-row quadrant: garbage rows x zero weights = NaN,
    so K=32 slices need their SBUF rows zeroed.
  - DVE memset/ops need 32-aligned partition base (walrus birverifier).
  - PSUM-source DMA is not allowed (DVE-copy to SBUF first).
  - ACT measured: ~0.49ns/col marginal + ~350-420ns/op fixed, dtype-indep;
    an op may span up to 4 contiguous PSUM banks (more banks don't fit).
  - DVE fp32 ~1.1ns/col + ~260ns fixed; DVE RECIPROCAL is ~10x slower
    (12us per 2048 cols) so a rational-tanh offload to DVE LOSES (tried:
    520us vs 259us).  The device duty-cycle throttles (ham k=4/8 windows,
    "activity" limit ~50%) under dense PE/DVE activity: adding 13 rank-1
    b2-fold matmuls/tile to merge ACT2 ops raised throttle 61->178us and
    lost 60us net (tried: 321us).  Keep total engine activity minimal.
  - bf16 matmul ~213ns warm at N=512 (doubles inside throttle windows).
  - Run-to-run variance: a hot device (back-to-back runs) adds ~20-50us;
    idle cooldown between runs restores ~255-259us.
  - Untried idea: xt DMA moves 4x the useful bytes (zeros padding); a
    dense [nt, 4, 48, TILE] layout + strided DMA needs the gap rows
    zeroed once per pool buffer (32-aligned memsets or a zeros-DMA), cut
    short by budget -- see transcript.
"""

import os
import sys

import numpy as np


def _ensure_path():
    for p in ("/opt/trn_rl_repo",):
        if p not in sys.path:
            sys.path.insert(0, p)


try:
    import concourse.bass as bass  # noqa: F401
except ImportError:
    _ensure_path()

import concourse.bass as bass  # noqa: F401
import concourse.bacc as bacc
import concourse.mybir as mybir
import concourse.tile as tile
from contextlib import ExitStack
from concourse.bass_utils import run_bass_kernel_spmd

NCORES = 8
B = 65536
BC = B // NCORES
T = 26
C = 5
H = 64
NPAIR = 13
TILE = 512
F32 = mybir.dt.float32
BF16 = mybir.dt.bfloat16
TANH = mybir.ActivationFunctionType.Tanh
ADD = mybir.AluOpType.add
MULT = mybir.AluOpType.mult
MIN = mybir.AluOpType.min
MAX = mybir.AluOpType.max

PLANE_PAIRS = [[0, 1, 2, 3], [4, 5, 6, 7], [8, 9, 10, 11], [12]]
DVE_PLANES = ()  # planes whose layer-1 tanh runs on the Vector engine

# Rational tanh fit on [-4.5, 4.5] (max abs err 1.9e-4 incl. saturation):
#   n = ((u+G1)*u+G0)*x ; d = ((u+D1)*u+D0)*KQ ; tanh ~ n/d ; u = clamp(x)^2
TG1, TG0 = 144.13813397, 1387.97534909
TD1, TD0 = 32.89048084, 75.28148013
TKQ = 18.43868257
TCLAMP = 4.5

# Drain counts per gap (after each plane step). Tuned so ScalarE stays fed
# while ps1 refills and (for DVE planes) the vector chain completes.
GAPS_SCALAR = [3, 3, 3, 4]  # DVE_PLANES = ()
GAPS_DVE1 = [3, 3, 1, 6]  # DVE_PLANES = ()

LAST_RESULTS = None


def build_nc(bc=BC):
    nt = bc // TILE
    nc = bacc.Bacc("TRN2", target_bir_lowering=False, debug=False)
    xt_d = nc.dram_tensor("xt", [nt, 4, 128, TILE], BF16, kind="ExternalInput")
    w1a_d = nc.dram_tensor("w1a", [128, NPAIR * 128], BF16, kind="ExternalInput")
    w2a_d = nc.dram_tensor("w2a", [128, NPAIR * 128], BF16, kind="ExternalInput")
    w3a_d = nc.dram_tensor("w3a", [128, NPAIR * 128], BF16, kind="ExternalInput")
    b2a_d = nc.dram_tensor("b2a", [128, NPAIR], F32, kind="ExternalInput")
    out = nc.dram_tensor("out", [T, bc], F32, kind="ExternalOutput")

    gaps = GAPS_DVE1 if DVE_PLANES == (1,) else GAPS_SCALAR

    with tile.TileContext(nc) as tc, ExitStack() as ctx:
        wpool = ctx.enter_context(tc.tile_pool(name="weights", bufs=1))
        xtpool = ctx.enter_context(tc.tile_pool(name="xt", bufs=3))
        h1pool = ctx.enter_context(tc.tile_pool(name="h1", bufs=6))
        h2pool = ctx.enter_context(tc.tile_pool(name="h2", bufs=4))
        s3pool = ctx.enter_context(tc.tile_pool(name="s3", bufs=2))
        dvpool = ctx.enter_context(tc.tile_pool(name="dv", bufs=1))
        pp1 = ctx.enter_context(tc.tile_pool(name="ps1", bufs=1, space="PSUM"))
        pp2 = ctx.enter_context(tc.tile_pool(name="ps2", bufs=2, space="PSUM"))
        pp3 = ctx.enter_context(tc.tile_pool(name="ps3", bufs=2, space="PSUM"))

        # warm the ACT tanh table while the setup DMAs run
        wrm = wpool.tile([1, 1], F32)
        nc.vector.memset(wrm, 0.0)
        nc.scalar.activation(out=wrm, in_=wrm, func=TANH)

        # setup DMAs: w1a first on the sync queue (layer 1 needs it first);
        # the rest ride the gpsimd DGE so tile 0's xt isn't queued behind.
        w1a = wpool.tile([128, NPAIR * 128], BF16)
        nc.sync.dma_start(out=w1a, in_=w1a_d[:, :])
        w2a = wpool.tile([128, NPAIR * 128], BF16)
        w3a = wpool.tile([128, NPAIR * 128], BF16)
        b2a = wpool.tile([128, NPAIR], F32)

        # ---- software-pipeline state ----
        fifo = []  # entries: (ready_key, tile_idx, pair, k, h1_handle)
        l3q = []  # pairs whose L3 matmul is deferred to the next slot
        ps3_state = {}  # tile_idx -> [ps3_handle, n_emitted]

        def emit_l2(ent):
            _, i, p, k, h1 = ent
            ps2 = pp2.tile([128, TILE], F32, tag="ps2")
            nc.tensor.matmul(
                out=ps2,
                lhsT=w2a[:, 128 * p : 128 * (p + 1)],
                rhs=h1[:, k, :],
                start=True,
                stop=True,
            )
            h2 = h2pool.tile([128, TILE], BF16, tag="h2")
            nc.scalar.activation(
                out=h2, in_=ps2, func=TANH, bias=b2a[:, p : p + 1], scale=1.0
            )
            l3q.append((i, p, h2))

        def emit_l3s():
            while l3q:
                i, p, h2 = l3q.pop(0)
                if i not in ps3_state:
                    ps3_state[i] = [
                        pp3.tile([128, TILE], F32, tag="ps3", name="ps3"),
                        0,
                    ]
                st = ps3_state[i]
                st[1] += 1
                nc.tensor.matmul(
                    out=st[0],
                    lhsT=w3a[:, 128 * p : 128 * (p + 1)],
                    rhs=h2,
                    start=(st[1] == 1),
                    stop=(st[1] == NPAIR),
                )
                if st[1] == NPAIR:
                    s3 = s3pool.tile([T, TILE], F32, tag="s3")
                    nc.vector.tensor_copy(out=s3, in_=st[0][0:T, :])
                    nc.gpsimd.dma_start(
                        out=out[:, i * TILE : (i + 1) * TILE], in_=s3
                    )
                    del ps3_state[i]

        def pop_ready(n, now):
            got = []
            j = 0
            while j < len(fifo) and len(got) < n:
                if fifo[j][0] <= now:
                    got.append(fifo.pop(j))
                else:
                    j += 1
            return got

        def emit_dve_tanh(ps1, h1, npr):
            xc = dvpool.tile([128, 4, TILE], F32, tag="dv_xc")
            u = dvpool.tile([128, 4, TILE], F32, tag="dv_u")
            a = dvpool.tile([128, 4, TILE], F32, tag="dv_a")
            b = dvpool.tile([128, 4, TILE], F32, tag="dv_b")
            xc, u, a, b = (z[:, 0:npr, :] for z in (xc, u, a, b))
            src = ps1[:, 0:npr, :]
            nc.vector.tensor_scalar(
                out=xc, in0=src, scalar1=-TCLAMP, scalar2=TCLAMP, op0=MAX, op1=MIN
            )
            nc.vector.tensor_mul(u, xc, xc)
            nc.vector.scalar_tensor_tensor(
                out=a, in0=u, scalar=TG1, in1=u, op0=ADD, op1=MULT
            )
            nc.vector.scalar_tensor_tensor(
                out=b, in0=a, scalar=TG0, in1=xc, op0=ADD, op1=MULT
            )
            nc.vector.scalar_tensor_tensor(
                out=a, in0=u, scalar=TD1, in1=u, op0=ADD, op1=MULT
            )
            nc.vector.tensor_scalar(
                out=u, in0=a, scalar1=TD0, scalar2=TKQ, op0=ADD, op1=MULT
            )
            nc.vector.reciprocal(out=a, in_=u)
            nc.vector.tensor_mul(h1[:, 0:npr, :], b, a)

        xt_tiles = {}

        def emit_xt_dma(i, split=False):
            xt = xtpool.tile([128, 4, TILE], BF16, tag="xt")
            if split:
                nc.gpsimd.dma_start(out=xt[:, 0, :], in_=xt_d[i, 0])
                nc.sync.dma_start(
                    out=xt[:, 1:4, :],
                    in_=xt_d[i, 1:4].rearrange("q p n -> p q n"),
                )
            else:
                nc.sync.dma_start(out=xt, in_=xt_d[i].rearrange("q p n -> p q n"))
            xt_tiles[i] = xt

        def emit_l1(i, q):
            prs = PLANE_PAIRS[q]
            ps1 = pp1.tile([128, 4, TILE], F32, tag="ps1")
            xt = xt_tiles[i]
            for k, p in enumerate(prs):
                nc.tensor.matmul(
                    out=ps1[:, k, :],
                    lhsT=w1a[32 * k : 32 * k + 32, 128 * p : 128 * (p + 1)],
                    rhs=xt[32 * k : 32 * k + 32, q, :],
                    start=True,
                    stop=True,
                    tile_position=(32 * k, 0),
                )
            return ps1

        # Slot pipeline over (tile, plane). Per slot: the activation for the
        # ps1 filled in the previous slot, then deferred L3s, two L2+ACT2
        # drains, the NEXT slot's L1 matmuls, and the remaining drains --
        # an order that keeps the in-order PE queue from stalling ScalarE.
        slots = [(i, q) for i in range(nt) for q in range(4)]
        emit_xt_dma(0, split=True)  # tile0 plane0 leads the gpsimd queue
        nc.gpsimd.dma_start(out=w2a, in_=w2a_d[:, :])
        nc.gpsimd.dma_start(out=w3a, in_=w3a_d[:, :])
        nc.gpsimd.dma_start(out=b2a, in_=b2a_d[:, :])
        ps1 = emit_l1(0, 0)
        for j, (i, q) in enumerate(slots):
            prs = PLANE_PAIRS[q]
            npr = len(prs)
            h1 = h1pool.tile([128, 4, TILE], BF16, tag="h1")
            if q in DVE_PLANES:
                emit_dve_tanh(ps1, h1, npr)
                ready = (i + 1, 0)  # drainable from next tile's slot 0
            else:
                nc.scalar.activation(
                    out=h1[:, 0:npr, :], in_=ps1[:, 0:npr, :], func=TANH
                )
                # plane q drains from the gap after plane q+1 (wrapping);
                # on the last tile drain immediately to shorten the tail
                if i == nt - 1:
                    ready = (i, q)
                else:
                    ready = (i, q + 1) if q < 3 else (i + 1, 0)
            for k, p in enumerate(prs):
                fifo.append((ready, i, p, k, h1))
            d = pop_ready(gaps[q] if i < nt - 1 else 6, (i, q))
            for ent in d[:2]:
                emit_l2(ent)
            if q == 1 and i + 1 < nt:
                emit_xt_dma(i + 1)
            if j + 1 < len(slots):
                ps1 = emit_l1(*slots[j + 1])
            for ent in d[2:]:
                emit_l2(ent)
            # L3s go LAST: they have a tile of slack, and ahead of the L1
            # matmuls they starve ScalarE's next ACT1 when PE is throttled
            emit_l3s()

        # flush the pipeline tail
        while fifo:
            emit_l3s()
            for ent in pop_ready(3, (nt + 1, 0)):
                emit_l2(ent)
        emit_l3s()
    return nc


def pack_weights(W1, b1, W2, b2, W3, b3):
    W1 = np.asarray(W1, dtype=np.float32)
    b1 = np.asarray(b1, dtype=np.float32)
    W2 = np.asarray(W2, dtype=np.float32)
    b2 = np.asarray(b2, dtype=np.float32)
    W3 = np.asarray(W3, dtype=np.float32)
    import ml_dtypes

    bf16 = ml_dtypes.bfloat16
    w1a = np.zeros((128, NPAIR * 128), np.float32)
    w2a = np.zeros((128, NPAIR * 128), np.float32)
    w3a = np.zeros((128, NPAIR * 128), np.float32)
    b2a = np.zeros((128, NPAIR), np.float32)
    for t in range(T):
        p, e = divmod(t, 2)
        k = p % 4
        w1a[32 * k + 5 * e : 32 * k + 5 * e + 5,
            128 * p + 64 * e : 128 * p + 64 * e + 64] = W1[t]
        w1a[32 * k + 10, 128 * p + 64 * e : 128 * p + 64 * e + 64] = b1[t]
        w2a[64 * e : 64 * e + 64, 128 * p + 64 * e : 128 * p + 64 * e + 64] = W2[t]
        # L3: pair p writes ps3 rows 2p, 2p+1 -> nonzero cols 2p+e in-block
        w3a[64 * e : 64 * e + 64, 128 * p + 2 * p + e] = W3[t][:, 0]
        b2a[64 * e : 64 * e + 64, p] = b2[t]
    return {
        "w1a": w1a.astype(bf16),
        "w2a": w2a.astype(bf16),
        "w3a": w3a.astype(bf16),
        "b2a": b2a,
    }


def pack_xt(features_core):
    """[bc, 26, 5] -> [nt, 4, 128, TILE] pair-aligned transposed layout."""
    bc = features_core.shape[0]
    nt = bc // TILE
    ff = np.asarray(features_core, np.float32).reshape(nt, TILE, T, C)
    import ml_dtypes

    xt = np.zeros((nt, 4, 128, TILE), ml_dtypes.bfloat16)
    for t in range(T):
        p, e = divmod(t, 2)
        q, k = divmod(p, 4)
        xt[:, q, 32 * k + 5 * e : 32 * k + 5 * e + 5, :] = ff[:, :, t, :].swapaxes(
            1, 2
        )
    for p in range(NPAIR):
        q, k = divmod(p, 4)
        xt[:, q, 32 * k + 2 * C, :] = 1.0
    return xt


def kernel(features, W1, b1, W2, b2, W3, b3):
    global LAST_RESULTS
    features = np.asarray(features, dtype=np.float32)
    b3 = np.asarray(b3, dtype=np.float32)
    ins = pack_weights(W1, b1, W2, b2, W3, b3)
    nc = build_nc(BC)
    nc.finalize()
    in_maps = []
    for c in range(NCORES):
        m = dict(ins)
        m["xt"] = pack_xt(features[c * BC : (c + 1) * BC])
        in_maps.append(m)
    trace = bool(int(os.environ.get("KERNEL_TRACE", "0")))
    # The first execution of a freshly loaded NEFF intermittently faults with
    # NRT_EXEC_UNIT_UNRECOVERABLE; a retry on the recovered device succeeds.
    last_exc = None
    for attempt in range(3):
        try:
            res = run_bass_kernel_spmd(
                nc, in_maps, list(range(NCORES)), trace=trace
            )
            LAST_RESULTS = res
            # out is type-major [26, bc]; transpose + b3 on the host
            return np.concatenate(
                [
                    res.results[c]["out"].T + b3[:, 0][None, :]
                    for c in range(NCORES)
                ],
                axis=0,
            )
        except Exception as e:  # noqa: BLE001
            last_exc = e
            import time as _time

            _time.sleep(5.0 * (attempt + 1))
    raise last_exc
